# revision 2
# baseline (speedup 1.0000x reference)
"""CGCNN-style GNN message passing on 8 Trainium2 NeuronCores (Bass/Tile).

Sharding: graphs partitioned 32/device => contiguous node ranges (batch is
sorted); edges assigned to the device owning their destination (col) and
sorted by col, so scatter-add is device-local. h is replicated each layer via
AllGather; BN statistics via a tiny AllReduce.

Math restructuring (all exact, host-side weight folds):
  per layer l:  s1 = softplus(phi1_l[row] + phi2_l[col] + p_l + b)       [edge]
                s2 = softplus(psi_l[row] + s1 @ Wx_l + bf)               [edge]
                p_{l+1} = s1 @ Wy_l (+ folded consts)                    [edge]
                h_new = scatter_col(s2) @ nu_W2 + deg * nu_b2            [node]
  where phi1 = h@eu_W1[:64], psi = h@nu_W1[:64], phi2 = h@eu_W1[64:128]
  are node-level projections (computed once per layer on each device from the
  replicated h and stored in DRAM tables), gathered per edge via indirect DMA.
  ea (edge features) never materialize: only their projection p flows.
"""
import os
import sys

sys.path.insert(0, "/opt/trn_rl_repo")
os.environ.setdefault("BASS_NEVER_TRACE", "1")

import numpy as np
import ml_dtypes

import concourse.bass as bass
import concourse.bacc as bacc
import concourse.mybir as mybir
import concourse.tile as tile
from concourse.masks import make_identity

N, E, G = 50000, 500000, 256
D = 64
NDEV, GPD = 8, 32
NP = 6528                 # padded nodes per device
W = NP // 128             # 50 windows per device
L = 3
EPS = 1e-5
P = 128
F32 = mybir.dt.float32
BF16 = mybir.dt.bfloat16
I32 = mybir.dt.int32
AF = mybir.ActivationFunctionType
BF = np.dtype(ml_dtypes.bfloat16)

_CACHE = {}


def _build_program(C):
    nc = bacc.Bacc("TRN2", target_bir_lowering=False, debug=False, num_devices=NDEV)
    f = nc.dram_tensor
    # per-device inputs
    xaugT = f("xaugT", [94, NP], F32, kind="ExternalInput").ap()
    rowt_all = f("rowt_all", [W * P, C], I32, kind="ExternalInput").ap()
    colt_all = f("colt_all", [W * P, C], I32, kind="ExternalInput").ap()
    creb_all = f("creb_all", [W * P, C], F32, kind="ExternalInput").ap()
    p_in = f("p_in", [W * P, C * D], F32, kind="ExternalInput").ap()
    breb_all = f("breb_all", [W * P, 1], F32, kind="ExternalInput").ap()
    invc = f("invc", [D, GPD], F32, kind="ExternalInput").ap()
    # shared params
    Waug = f("Waug", [94, D], F32, kind="ExternalInput").ap()
    Wstack3 = f("Wstack3", [L, D, 192], F32, kind="ExternalInput").ap()
    bias3 = f("bias3", [L, P, 192], F32, kind="ExternalInput").ap()
    Wxy3 = f("Wxy3", [L, D, P], F32, kind="ExternalInput").ap()
    W2b3 = f("W2b3", [L, D + 1, D], F32, kind="ExternalInput").ap()
    gb3 = f("gb3", [L, D, 2], F32, kind="ExternalInput").ap()
    pW1 = f("pW1", [D, P], F32, kind="ExternalInput").ap()
    pb1 = f("pb1", [P, 1], F32, kind="ExternalInput").ap()
    pW2 = f("pW2", [P, P], F32, kind="ExternalInput").ap()
    pb2 = f("pb2", [P, 1], F32, kind="ExternalInput").ap()
    pW3 = f("pW3", [P, 1], F32, kind="ExternalInput").ap()
    pb3 = f("pb3", [1, 1], F32, kind="ExternalInput").ap()
    out32 = f("out32", [1, GPD], F32, kind="ExternalOutput").ap()
    dbg_h0 = f("dbg_h0", [D, NP], F32, kind="ExternalOutput").ap()
    dbg_hn0 = f("dbg_hn0", [D, NP], F32, kind="ExternalOutput").ap()
    dbg_st = f("dbg_st", [L, D, 2], F32, kind="ExternalOutput").ap()
    dbg_h1 = f("dbg_h1", [D, NP], F32, kind="ExternalOutput").ap()
    # internal DRAM
    PhiR = f("PhiR", [NDEV * NP, P], F32).ap()
    PhiC = f("PhiC", [NDEV * NP, D], F32).ap()
    hT_loc = f("hT_loc", [D, NP], F32).ap()
    h_new = f("h_new", [D, NP], F32).ap()
    p_cur = f("p_cur", [W * P, C * D], F32).ap()
    ag_src = f("ag_src", [D, NP], F32).ap()
    ag_dst = f("ag_dst", [NDEV * D, NP], F32, addr_space="Shared").ap()
    st_src = f("st_src", [D, 2], F32).ap()
    st_dst = f("st_dst", [D, 2], F32, addr_space="Shared").ap()
    rg = [list(range(NDEV))]

    with tile.TileContext(nc) as tc:
        with tc.tile_pool(name="const", bufs=1) as cp:
            ident = cp.tile([P, P], F32)
            make_identity(nc, ident[:])
            iota = cp.tile([P, P], F32)
            nc.gpsimd.iota(iota[:], pattern=[[1, P]], base=0, channel_multiplier=0,
                           allow_small_or_imprecise_dtypes=True)
            waug_sb = cp.tile([94, D], F32)
            nc.sync.dma_start(out=waug_sb[:], in_=Waug)
            ws_sb, bi_sb, wxy_sb, w2b_sb, gb_sb = [], [], [], [], []
            for l in range(L):
                t = cp.tile([D, 192], F32, name=f"ws{l}")
                nc.sync.dma_start(out=t[:], in_=Wstack3[l])
                ws_sb.append(t)
                t = cp.tile([P, 192], F32, name=f"bi{l}")
                nc.sync.dma_start(out=t[:], in_=bias3[l])
                bi_sb.append(t)
                t = cp.tile([D, P], F32, name=f"wxy{l}")
                nc.sync.dma_start(out=t[:], in_=Wxy3[l])
                wxy_sb.append(t)
                t = cp.tile([D + 1, D], F32, name=f"w2b{l}")
                nc.sync.dma_start(out=t[:], in_=W2b3[l])
                w2b_sb.append(t)
                t = cp.tile([D, 2], F32, name=f"gb{l}")
                nc.sync.dma_start(out=t[:], in_=gb3[l])
                gb_sb.append(t)
            invc_sb = cp.tile([D, GPD], F32)
            nc.sync.dma_start(out=invc_sb[:], in_=invc)
            pw1_sb = cp.tile([D, P], F32)
            nc.sync.dma_start(out=pw1_sb[:], in_=pW1)
            pb1_sb = cp.tile([P, 1], F32)
            nc.sync.dma_start(out=pb1_sb[:], in_=pb1)
            pw2_sb = cp.tile([P, P], F32)
            nc.sync.dma_start(out=pw2_sb[:], in_=pW2)
            pb2_sb = cp.tile([P, 1], F32)
            nc.sync.dma_start(out=pb2_sb[:], in_=pb2)
            pw3_sb = cp.tile([P, 1], F32)
            nc.sync.dma_start(out=pw3_sb[:], in_=pW3)
            pb3_sb = cp.tile([1, 1], F32)
            nc.sync.dma_start(out=pb3_sb[:], in_=pb3)

            # ---- embed + p init ----
            with tc.tile_pool(name="emb", bufs=3) as ep, \
                 tc.tile_pool(name="embp", bufs=2, space="PSUM") as epp:
                for w in range(W):
                    xt = ep.tile([94, P], F32, tag="xt")
                    nc.sync.dma_start(out=xt[:], in_=xaugT[:, w * P:(w + 1) * P])
                    ph = epp.tile([D, P], F32, tag="eps")
                    nc.tensor.matmul(out=ph[:], lhsT=waug_sb[:], rhs=xt[:], start=True, stop=True)
                    hs = ep.tile([D, P], F32, tag="hs")
                    nc.vector.tensor_copy(out=hs[:], in_=ph[:])
                    nc.sync.dma_start(out=hT_loc[:, w * P:(w + 1) * P], in_=hs[:])
                    nc.sync.dma_start(out=dbg_h0[:, w * P:(w + 1) * P], in_=hs[:])
                    hb = ep.tile([D, P], F32, tag="hb")
                    nc.vector.tensor_copy(out=hb[:], in_=ph[:])
                    nc.sync.dma_start(out=ag_src[:, w * P:(w + 1) * P], in_=hb[:])
                    pc = ep.tile([P, C * D], F32, tag="pc")
                    nc.sync.dma_start(out=pc[:], in_=p_in[w * P:(w + 1) * P, :])
                    nc.sync.dma_start(out=p_cur[w * P:(w + 1) * P, :], in_=pc[:])

            for l in range(L):
                nc.gpsimd.collective_compute(
                    "AllGather", mybir.AluOpType.bypass, replica_groups=rg,
                    ins=[ag_src], outs=[ag_dst])
                # ---- phi production ----
                with tc.tile_pool(name=f"phi{l}", bufs=3) as pp, \
                     tc.tile_pool(name=f"phip{l}", bufs=2, space="PSUM") as ppp:
                    def phi_body(iv, l=l, pp=pp, ppp=ppp):
                        for d in range(NDEV):
                            ht = pp.tile([D, P], F32, tag="ht")
                            nc.sync.dma_start(out=ht[:], in_=ag_dst[d * D:(d + 1) * D, bass.ts(iv, P)])
                            ps = ppp.tile([P, 192], F32, tag="pps")
                            nc.tensor.matmul(out=ps[:], lhsT=ht[:], rhs=ws_sb[l][:], start=True, stop=True)
                            phb = pp.tile([P, 192], F32, tag="phb")
                            nc.vector.tensor_add(out=phb[:], in0=ps[:], in1=bi_sb[l][:])
                            nc.sync.dma_start(out=PhiR[bass.ds(iv * P + d * NP, P), :], in_=phb[:, :P])
                            nc.sync.dma_start(out=PhiC[bass.ds(iv * P + d * NP, P), :], in_=phb[:, P:192])
                    tc.For_i_unrolled(0, W, 1, phi_body, max_unroll=4)
                # ---- edge phase ----
                with tc.tile_pool(name=f"lay{l}", bufs=1) as lp, \
                     tc.tile_pool(name=f"wst{l}", bufs=2) as wp, \
                     tc.tile_pool(name=f"work{l}", bufs=3) as sp, \
                     tc.tile_pool(name=f"pscat{l}", bufs=1, space="PSUM") as pw_pool, \
                     tc.tile_pool(name=f"pt{l}", bufs=2, space="PSUM") as pt_pool, \
                     tc.tile_pool(name=f"pf{l}", bufs=2, space="PSUM") as pf_pool, \
                     tc.tile_pool(name=f"ph{l}", bufs=1, space="PSUM") as ph_pool:
                    sS = lp.tile([D, 1], F32, name=f"sS{l}")
                    sQ = lp.tile([D, 1], F32, name=f"sQ{l}")
                    nc.vector.memset(sS[:], 0.0)
                    nc.vector.memset(sQ[:], 0.0)

                    def edge_body(iv, l=l, sS=sS, sQ=sQ, wp=wp, sp=sp,
                                  pw_pool=pw_pool, pt_pool=pt_pool, pf_pool=pf_pool, ph_pool=ph_pool):
                        rw = wp.tile([P, C], I32, tag="rw")
                        nc.sync.dma_start(out=rw[:], in_=rowt_all[bass.ts(iv, P), :])
                        cw = wp.tile([P, C], I32, tag="cw")
                        nc.sync.dma_start(out=cw[:], in_=colt_all[bass.ts(iv, P), :])
                        cb = wp.tile([P, C], F32, tag="cb")
                        nc.sync.dma_start(out=cb[:], in_=creb_all[bass.ts(iv, P), :])
                        pwt = wp.tile([P, C * D], F32, tag="pwt")
                        nc.sync.dma_start(out=pwt[:], in_=p_cur[bass.ts(iv, P), :])
                        scat = pw_pool.tile([D + 1, P], F32, tag="scat")
                        for s in range(C):
                            gA = sp.tile([P, P], F32, tag="gA")
                            nc.gpsimd.indirect_dma_start(
                                out=gA[:], out_offset=None, in_=PhiR[:, :],
                                in_offset=bass.IndirectOffsetOnAxis(ap=rw[:, s:s + 1], axis=0))
                            gC = sp.tile([P, D], F32, tag="gC")
                            nc.gpsimd.indirect_dma_start(
                                out=gC[:], out_offset=None, in_=PhiC[:, :],
                                in_offset=bass.IndirectOffsetOnAxis(ap=cw[:, s:s + 1], axis=0))
                            s1p = sp.tile([P, D], F32, tag="s1p")
                            nc.vector.tensor_add(out=s1p[:], in0=gA[:, :D], in1=gC[:])
                            nc.vector.tensor_add(out=s1p[:], in0=s1p[:], in1=pwt[:, s * D:(s + 1) * D])
                            s1tp = pt_pool.tile([D, P], F32, tag="t")
                            nc.tensor.transpose(out=s1tp[:], in_=s1p[:], identity=ident[:])
                            s1e = sp.tile([D, P], F32, tag="s1e")
                            nc.scalar.activation(out=s1e[:], in_=s1tp[:], func=AF.Exp)
                            s1t = sp.tile([D, P], F32, tag="s1t")
                            nc.scalar.activation(out=s1t[:], in_=s1e[:], func=AF.Ln, bias=1.0)
                            fm = pf_pool.tile([P, P], F32, tag="fm")
                            nc.tensor.matmul(out=fm[:], lhsT=s1t[:], rhs=wxy_sb[l][:], start=True, stop=True)
                            nc.vector.tensor_copy(out=pwt[:, s * D:(s + 1) * D], in_=fm[:, D:])
                            s2p = sp.tile([P, D], F32, tag="s2p")
                            nc.vector.tensor_add(out=s2p[:], in0=gA[:, D:P], in1=fm[:, :D])
                            s2e = sp.tile([P, D], F32, tag="s2e")
                            nc.scalar.activation(out=s2e[:], in_=s2p[:], func=AF.Exp)
                            s2t = sp.tile([P, D + 1], F32, tag="s2t")
                            nc.scalar.activation(out=s2t[:, :D], in_=s2e[:], func=AF.Ln, bias=1.0)
                            nc.vector.memset(s2t[:, D:D + 1], 1.0)
                            sel = sp.tile([P, P], F32, tag="sel")
                            nc.vector.tensor_tensor(out=sel[:], in0=cb[:, s:s + 1].to_broadcast([P, P]),
                                                    in1=iota[:], op=mybir.AluOpType.is_equal)
                            nc.tensor.matmul(out=scat[:], lhsT=s2t[:], rhs=sel[:],
                                             start=(s == 0), stop=(s == C - 1))
                        nc.sync.dma_start(out=p_cur[bass.ts(iv, P), :], in_=pwt[:])
                        pws = sp.tile([D + 1, P], F32, tag="pws")
                        nc.vector.tensor_copy(out=pws[:], in_=scat[:])
                        hn = ph_pool.tile([D, P], F32, tag="hn")
                        nc.tensor.matmul(out=hn[:], lhsT=w2b_sb[l][:], rhs=pws[:], start=True, stop=True)
                        r1 = sp.tile([D, 1], F32, tag="r1")
                        nc.vector.reduce_sum(out=r1[:], in_=hn[:], axis=mybir.AxisListType.X)
                        nc.vector.tensor_add(out=sS[:], in0=sS[:], in1=r1[:])
                        sq = sp.tile([D, P], F32, tag="sq")
                        nc.scalar.activation(out=sq[:], in_=hn[:], func=AF.Square)
                        r2 = sp.tile([D, 1], F32, tag="r2")
                        nc.vector.reduce_sum(out=r2[:], in_=sq[:], axis=mybir.AxisListType.X)
                        nc.vector.tensor_add(out=sQ[:], in0=sQ[:], in1=r2[:])
                        hws = sp.tile([D, P], F32, tag="hws")
                        nc.vector.tensor_copy(out=hws[:], in_=hn[:])
                        nc.sync.dma_start(out=h_new[:, bass.ts(iv, P)], in_=hws[:])
                        if l == 0:
                            nc.sync.dma_start(out=dbg_hn0[:, bass.ts(iv, P)], in_=hws[:])
                    tc.For_i_unrolled(0, W, 1, edge_body, max_unroll=2)

                    nc.sync.dma_start(out=st_src[:, 0:1], in_=sS[:])
                    nc.sync.dma_start(out=st_src[:, 1:2], in_=sQ[:])
                nc.gpsimd.collective_compute(
                    "AllReduce", mybir.AluOpType.add, replica_groups=rg,
                    ins=[st_src], outs=[st_dst])
                # ---- BN apply + residual ----
                with tc.tile_pool(name=f"bn{l}", bufs=3) as bp:
                    sts = bp.tile([D, 2], F32, tag="sts")
                    nc.sync.dma_start(out=sts[:], in_=st_dst)
                    nc.sync.dma_start(out=dbg_st[l], in_=sts[:])
                    mu = bp.tile([D, 1], F32, tag="mu")
                    nc.vector.tensor_scalar_mul(out=mu[:], in0=sts[:, 0:1], scalar1=1.0 / N)
                    msq = bp.tile([D, 1], F32, tag="msq")
                    nc.vector.tensor_scalar_mul(out=msq[:], in0=sts[:, 1:2], scalar1=1.0 / N)
                    mu2 = bp.tile([D, 1], F32, tag="mu2")
                    nc.vector.tensor_tensor(out=mu2[:], in0=mu[:], in1=mu[:], op=mybir.AluOpType.mult)
                    var = bp.tile([D, 1], F32, tag="var")
                    nc.vector.tensor_tensor(out=var[:], in0=msq[:], in1=mu2[:], op=mybir.AluOpType.subtract)
                    epsT = bp.tile([D, 1], F32, tag="epsT")
                    nc.vector.memset(epsT[:], EPS)
                    nc.vector.tensor_add(out=var[:], in0=var[:], in1=epsT[:])
                    lnv = bp.tile([D, 1], F32, tag="lnv")
                    nc.scalar.activation(out=lnv[:], in_=var[:], func=AF.Ln)
                    sc0 = bp.tile([D, 1], F32, tag="sc0")
                    nc.scalar.activation(out=sc0[:], in_=lnv[:], func=AF.Exp, scale=-0.5)
                    scal = bp.tile([D, 1], F32, tag="scal")
                    nc.vector.tensor_tensor(out=scal[:], in0=sc0[:], in1=gb_sb[l][:, 0:1], op=mybir.AluOpType.mult)
                    msc = bp.tile([D, 1], F32, tag="msc")
                    nc.vector.tensor_tensor(out=msc[:], in0=mu[:], in1=scal[:], op=mybir.AluOpType.mult)
                    shift = bp.tile([D, 1], F32, tag="shift")
                    nc.vector.tensor_tensor(out=shift[:], in0=gb_sb[l][:, 1:2], in1=msc[:], op=mybir.AluOpType.subtract)
                    CH = 544
                    for k in range(NP // CH):
                        hnk = bp.tile([D, CH], F32, tag="hnk")
                        nc.sync.dma_start(out=hnk[:], in_=h_new[:, k * CH:(k + 1) * CH])
                        bnk = bp.tile([D, CH], F32, tag="bnk")
                        nc.vector.tensor_scalar(out=bnk[:], in0=hnk[:], scalar1=scal[:, 0:1],
                                                scalar2=shift[:, 0:1], op0=mybir.AluOpType.mult,
                                                op1=mybir.AluOpType.add)
                        ek = bp.tile([D, CH], F32, tag="ek")
                        nc.scalar.activation(out=ek[:], in_=bnk[:], func=AF.Exp)
                        spk = bp.tile([D, CH], F32, tag="spk")
                        nc.scalar.activation(out=spk[:], in_=ek[:], func=AF.Ln, bias=1.0)
                        hok = bp.tile([D, CH], F32, tag="hok")
                        nc.sync.dma_start(out=hok[:], in_=hT_loc[:, k * CH:(k + 1) * CH])
                        nc.vector.tensor_add(out=hok[:], in0=hok[:], in1=spk[:])
                        nc.sync.dma_start(out=hT_loc[:, k * CH:(k + 1) * CH], in_=hok[:])
                        if l == 0:
                            nc.sync.dma_start(out=dbg_h1[:, k * CH:(k + 1) * CH], in_=hok[:])
                        if l < L - 1:
                            hbk = bp.tile([D, CH], F32, tag="hbk")
                            nc.vector.tensor_copy(out=hbk[:], in_=hok[:])
                            nc.sync.dma_start(out=ag_src[:, k * CH:(k + 1) * CH], in_=hbk[:])

            # ---- pooling + predictor ----
            with tc.tile_pool(name="pool", bufs=3) as qp, \
                 tc.tile_pool(name="poolp", bufs=1, space="PSUM") as pq, \
                 tc.tile_pool(name="ptr", bufs=2, space="PSUM") as ptr, \
                 tc.tile_pool(name="pz", bufs=1, space="PSUM") as pz:
                pooled = pq.tile([D, GPD], F32, tag="pooled")
                for w in range(W):
                    htk = qp.tile([D, P], F32, tag="htk")
                    nc.sync.dma_start(out=htk[:], in_=hT_loc[:, w * P:(w + 1) * P])
                    hnm = ptr.tile([P, D], F32, tag="tr")
                    nc.tensor.transpose(out=hnm[:], in_=htk[:], identity=ident[:D, :D])
                    hns = qp.tile([P, D], F32, tag="hns")
                    nc.vector.tensor_copy(out=hns[:], in_=hnm[:])
                    brw = qp.tile([P, 1], F32, tag="brw")
                    nc.sync.dma_start(out=brw[:], in_=breb_all[w * P:(w + 1) * P, :])
                    selg = qp.tile([P, GPD], F32, tag="selg")
                    nc.vector.tensor_tensor(out=selg[:], in0=brw[:, 0:1].to_broadcast([P, GPD]),
                                            in1=iota[:, :GPD], op=mybir.AluOpType.is_equal)
                    nc.tensor.matmul(out=pooled[:], lhsT=hns[:], rhs=selg[:],
                                     start=(w == 0), stop=(w == W - 1))
                reprT = qp.tile([D, GPD], F32, tag="reprT")
                nc.vector.tensor_copy(out=reprT[:], in_=pooled[:])
                nc.vector.tensor_tensor(out=reprT[:], in0=reprT[:], in1=invc_sb[:], op=mybir.AluOpType.mult)
                z1ps = pz.tile([P, GPD], F32, tag="z1")
                nc.tensor.matmul(out=z1ps[:], lhsT=pw1_sb[:], rhs=reprT[:], start=True, stop=True)
                z1e = qp.tile([P, GPD], F32, tag="z1e")
                nc.scalar.activation(out=z1e[:], in_=z1ps[:], func=AF.Exp, bias=pb1_sb[:, 0:1])
                z1 = qp.tile([P, GPD], F32, tag="z1s")
                nc.scalar.activation(out=z1[:], in_=z1e[:], func=AF.Ln, bias=1.0)
                z2ps = pz.tile([P, GPD], F32, tag="z2")
                nc.tensor.matmul(out=z2ps[:], lhsT=pw2_sb[:], rhs=z1[:], start=True, stop=True)
                z2e = qp.tile([P, GPD], F32, tag="z2e")
                nc.scalar.activation(out=z2e[:], in_=z2ps[:], func=AF.Exp, bias=pb2_sb[:, 0:1])
                z2 = qp.tile([P, GPD], F32, tag="z2s")
                nc.scalar.activation(out=z2[:], in_=z2e[:], func=AF.Ln, bias=1.0)
                ops = pz.tile([1, GPD], F32, tag="op")
                nc.tensor.matmul(out=ops[:], lhsT=pw3_sb[:], rhs=z2[:], start=True, stop=True)
                outs = qp.tile([1, GPD], F32, tag="outs")
                nc.vector.tensor_scalar(out=outs[:], in0=ops[:], scalar1=pb3_sb[:, 0:1],
                                        scalar2=None, op0=mybir.AluOpType.add)
                nc.sync.dma_start(out=out32, in_=outs[:])
    nc.compile()
    return nc


def _host_prep(inputs):
    x = np.asarray(inputs["x"], np.float32)
    edge_attr = np.asarray(inputs["edge_attr"], np.float32)
    charge = np.asarray(inputs["charge"], np.float32)
    ei = np.asarray(inputs["edge_index"], np.int64)
    batch = np.asarray(inputs["batch"], np.int64)
    g = lambda k: np.asarray(inputs[k], np.float32)
    euW1, eub1 = g("eu_W1"), g("eu_b1")
    euW2, eub2 = g("eu_W2"), g("eu_b2")
    nuW1, nub1 = g("nu_W1"), g("nu_b1")
    nuW2, nub2 = g("nu_W2"), g("nu_b2")

    row, col = ei[0], ei[1]
    nb = np.searchsorted(batch, np.arange(0, G + 1, GPD) - 0.5).astype(np.int64)
    nb[0], nb[-1] = 0, N
    node_ids = np.arange(N)
    node_dev = np.searchsorted(nb[1:], node_ids, side="right")
    node_tr = (node_dev * NP + (node_ids - nb[node_dev])).astype(np.int64)
    order = np.argsort(col, kind="stable")
    col_s, row_s = col[order], row[order]
    ea_s = edge_attr[order]
    eb = np.searchsorted(col_s, nb)

    # p1 = edge_attr @ (W_bond @ eu_W1[0][128:]) + b_bond @ eu_W1[0][128:]
    Wb1 = g("W_bond") @ euW1[0][128:192]
    cb1 = g("b_bond") @ euW1[0][128:192]
    p1_all = (ea_s @ Wb1 + cb1).astype(np.float32)

    # per-device window/slot structure
    devs = []
    C = 1
    for d in range(NDEV):
        cl = (col_s[eb[d]:eb[d + 1]] - nb[d]).astype(np.int64)
        n_loc = nb[d + 1] - nb[d]
        assert n_loc <= NP
        wb = np.searchsorted(cl, np.arange(W + 1) * P)
        deg = np.diff(wb)
        C = max(C, int(np.ceil(deg.max() / P)))
        devs.append((cl, wb, n_loc))

    counts = np.bincount(batch, minlength=G).astype(np.float32)
    invc_full = 1.0 / np.maximum(counts, 1.0)

    in_maps = []
    for d in range(NDEV):
        cl, wb, n_loc = devs[d]
        rows_d = row_s[eb[d]:eb[d + 1]]
        p1_d = p1_all[eb[d]:eb[d + 1]]
        ne = len(cl)
        win = cl >> 7
        j = np.arange(ne) - wb[win]
        slot, lane = j // P, j % P
        rowt = np.zeros((W, P, C), np.int32)
        colt = np.zeros((W, P, C), np.int32)
        creb = np.full((W, P, C), -1.0, np.float32)
        p1a = np.zeros((W, P, C * D), np.float32)
        rowt[win, lane, slot] = node_tr[rows_d]
        colt[win, lane, slot] = d * NP + cl
        creb[win, lane, slot] = (cl - (win << 7)).astype(np.float32)
        sl64 = (slot * D)[:, None] + np.arange(D)[None, :]
        p1a[win[:, None].repeat(D, 1), lane[:, None].repeat(D, 1), sl64] = p1_d

        breb = np.full(W * P, -1.0, np.float32)
        bl = (batch[nb[d]:nb[d + 1]] - d * GPD).astype(np.float32)
        breb[:n_loc] = bl
        xaugT = np.zeros((94, NP), np.float32)
        xaugT[:92, :n_loc] = x[nb[d]:nb[d + 1]].T
        xaugT[92, :n_loc] = charge[batch[nb[d]:nb[d + 1]]]
        xaugT[93, :n_loc] = 1.0
        invc_t = np.tile(invc_full[d * GPD:(d + 1) * GPD][None, :], (D, 1)).astype(np.float32)
        in_maps.append(dict(
            xaugT=xaugT, rowt_all=rowt.reshape(W * P, C), colt_all=colt.reshape(W * P, C),
            creb_all=creb.reshape(W * P, C), p_in=p1a.reshape(W * P, C * D),
            breb_all=breb.reshape(W * P, 1), invc=invc_t))

    # shared folded params
    Waug = np.zeros((94, D), np.float32)
    Waug[:92] = g("W_atom")[:92]
    Waug[92] = (g("W_charge") @ g("W_atom")[92:108]).reshape(D)
    Waug[93] = g("b_atom") + g("b_charge") @ g("W_atom")[92:108]
    Wstack3 = np.zeros((L, D, 192), np.float32)
    bias3 = np.zeros((L, 192), np.float32)
    Wxy3 = np.zeros((L, D, P), np.float32)
    W2b3 = np.zeros((L, D + 1, D), np.float32)
    gb3 = np.zeros((L, D, 2), np.float32)
    for l in range(L):
        Wstack3[l] = np.concatenate([euW1[l][:D], nuW1[l][:D], euW1[l][D:2 * D]], axis=1)
        bias3[l][:D] = eub1[l] + (eub2[l - 1] @ euW1[l][128:192] if l >= 1 else 0)
        bias3[l][D:2 * D] = nub1[l] + eub2[l] @ nuW1[l][D:2 * D]
        Wxy3[l][:, :D] = euW2[l] @ nuW1[l][D:2 * D]
        if l < L - 1:
            Wxy3[l][:, D:] = euW2[l] @ euW1[l + 1][128:192]
        W2b3[l][:D] = nuW2[l]
        W2b3[l][D] = nub2[l]
        gb3[l][:, 0] = g("bn_gamma")[l]
        gb3[l][:, 1] = g("bn_beta")[l]
    shared = dict(
        Waug=Waug, Wstack3=Wstack3, bias3=np.tile(bias3[:, None, :], (1, P, 1)),
        Wxy3=Wxy3, W2b3=W2b3, gb3=gb3,
        pW1=g("p_W1"), pb1=g("p_b1").reshape(P, 1), pW2=g("p_W2"),
        pb2=g("p_b2").reshape(P, 1), pW3=g("p_W3").reshape(P, 1),
        pb3=g("p_b3").reshape(1, 1))
    for m in in_maps:
        m.update(shared)
    return in_maps, C


LAST_RESULTS = None


def kernel(**inputs):
    global LAST_RESULTS
    in_maps, C = _host_prep(inputs)
    if C not in _CACHE:
        _CACHE[C] = _build_program(C)
    nc = _CACHE[C]
    from concourse.bass_utils import run_bass_kernel_spmd
    res = run_bass_kernel_spmd(nc, in_maps, core_ids=list(range(NDEV)))
    LAST_RESULTS = res
    out = np.concatenate([res.results[d]["out32"][0] for d in range(NDEV)])
    return out.astype(np.float32)

